# revision 2
# baseline (speedup 1.0000x reference)
"""Trainium2 Bass kernel for nn_CustomLlamaModel (2-layer MQA llama, B=1 S=2048
H=1024 HQ=16 HKV=1 FF=4096), fully token-sharded data-parallel over 8 cores.

Each core owns 256 tokens end-to-end; all weights are replicated (streamed
from HBM, overlapped with compute).  The only per-layer collective is a tiny
k/v AllGather (64KB in / 512KB out per core): k,v are computed from the local
token shard, k roped locally, v transposed to token-major, gathered, and
reassembled (k feature-major [64,2048] lhsT for scoresT, v token-major
[128,16x65] with a folded ones column so the softmax denominator rides the
attn@v matmul).  No 4MB AllGathers / ReduceScatters, no collective dead
windows.  Attention runs per 4-head group with N=512 matmuls, software-
pipelined so the PE never waits on the Exp activation.  MLP streams FF in
512-wide chunks, gate/up one step ahead of the down accumulation.  ln1/ln2
and the 1/sqrt(D) scale are folded into weights host-side; the embedding
gather runs host-side.
"""
import os
import sys

sys.path.insert(0, "/opt/trn_rl_repo")

import ml_dtypes
import numpy as np
import orjson

import concourse.bass as bass
import concourse.mybir as mybir
import concourse.tile as tile
from concourse import bass_utils
from concourse.masks import make_identity

# ---------------------------------------------------------------------------
# Walrus in this container supports only ONE sync-wait per instruction, but
# Tile's scheduler emits multi-wait instructions.  Post-process the BIR JSON:
# split each multi-wait instruction into single-wait NoOps (same engine,
# program-order before the original).
# ---------------------------------------------------------------------------
_orig_to_json_bytes = bass.Bass.to_json_bytes
_MW = [0]


def _split_multiwait(d):
    changed = False

    def fix_block(bb):
        nonlocal changed
        insts = bb.get("instructions")
        if not insts:
            return
        out = []
        for ins in insts:
            si = ins.get("sync_info")
            if si:
                ow = si.get("on_wait") or []
                if len(ow) > 1:
                    changed = True
                    for w in ow[:-1]:
                        _MW[0] += 1
                        out.append({
                            "debug": ins.get("debug", 0),
                            "engine": ins["engine"],
                            "ins": [],
                            "outs": [],
                            "name": f"{ins['name']}-mw{_MW[0]}",
                            "opcode": "NoOp",
                            "sync_info": {"on_update": [], "on_wait": [w]},
                        })
                    si["on_wait"] = [ow[-1]]
            out.append(ins)
        bb["instructions"] = out

    def rec(o):
        if isinstance(o, dict):
            if isinstance(o.get("instructions"), list):
                fix_block(o)
            for v in o.values():
                rec(v)
        elif isinstance(o, list):
            for v in o:
                rec(v)

    for fn in d.get("functions", []):
        rec(fn)
    return changed


def _patched_to_json_bytes(self):
    raw = _orig_to_json_bytes(self)
    d = orjson.loads(raw)
    if _split_multiwait(d):
        return orjson.dumps(d)
    return raw


bass.Bass.to_json_bytes = _patched_to_json_bytes

# ---------------------------------------------------------------------------
# Model / sharding constants
# ---------------------------------------------------------------------------
S, H, D, HQ, FF, L, V = 2048, 1024, 64, 16, 4096, 2, 32000
EPS = 1e-6
NCORES = 8
TOK = S // NCORES       # tokens per core (256)
HT = H // 128           # 8 hidden feature tiles
KT = S // 128           # 16 key-token tiles
MT = HQ // 2            # 8 q m-tiles (2 heads each)
FC = 8                  # FF chunks
FCW = FF // FC          # 512
FM = FCW // 128         # 4 sub-tiles per chunk
F32 = mybir.dt.float32
F32R = mybir.dt.float32r
BF16 = mybir.dt.bfloat16
MUL = mybir.AluOpType.mult
ADD = mybir.AluOpType.add
RG = [list(range(NCORES))]
BF = ml_dtypes.bfloat16

DEBUG = os.environ.get("KDP_DEBUG") == "1"
_CACHED_NC = None


def _build_nc():
    nc = bass.Bass()
    x0 = nc.dram_tensor("x0", [128, HT * TOK], F32, kind="ExternalInput")
    cosk = nc.dram_tensor("cosk", [64, TOK], BF16, kind="ExternalInput")
    sink = nc.dram_tensor("sink", [64, TOK], BF16, kind="ExternalInput")
    cosq = nc.dram_tensor("cosq", [64, 8 * TOK], BF16, kind="ExternalInput")
    sinq = nc.dram_tensor("sinq", [64, 8 * TOK], BF16, kind="ExternalInput")
    seld = nc.dram_tensor("sel", [16, MT * 128], F32, kind="ExternalInput")
    W = []
    for l in range(L):
        w = {
            "wkv": nc.dram_tensor(f"wkv{l}", [128, HT * 128], BF16,
                                  kind="ExternalInput"),
            "wq": nc.dram_tensor(f"wq{l}", [128, MT * HT * 128], BF16,
                                 kind="ExternalInput"),
            "wo": nc.dram_tensor(f"wo{l}", [128, HT * HT * 128], BF16,
                                 kind="ExternalInput"),
        }
        for fc in range(FC):
            w[f"wg{fc}"] = nc.dram_tensor(f"wg{l}_{fc}", [128, FM * HT * 128],
                                          BF16, kind="ExternalInput")
            w[f"wu{fc}"] = nc.dram_tensor(f"wu{l}_{fc}", [128, FM * HT * 128],
                                          BF16, kind="ExternalInput")
            w[f"wd{fc}"] = nc.dram_tensor(f"wd{l}_{fc}", [128, HT * FM * 128],
                                          BF16, kind="ExternalInput")
        W.append(w)
    xout = nc.dram_tensor("xout", [128, HT * TOK], F32, kind="ExternalOutput")
    dbg = {}
    if DEBUG:
        for nm, shp, dt in [
            ("dbg_h", [128, HT * TOK], BF16),
            ("dbg_q", [64, HQ * TOK], BF16),
            ("dbg_kT", [64, S], BF16),
            ("dbg_vt", [128, KT * 65], BF16),
            ("dbg_den", [1, HQ * TOK], F32),
            ("dbg_oT", [128, HT * TOK], BF16),
            ("dbg_x1", [128, HT * TOK], F32),
            ("dbg_h2", [128, HT * TOK], BF16),
            ("dbg_x2", [128, HT * TOK], F32),
            ("dbg_h_1", [128, HT * TOK], BF16),
            ("dbg_q_1", [64, HQ * TOK], BF16),
            ("dbg_kT_1", [64, S], BF16),
            ("dbg_vt_1", [128, KT * 65], BF16),
            ("dbg_den_1", [1, HQ * TOK], F32),
            ("dbg_oT_1", [128, HT * TOK], BF16),
            ("dbg_x1_1", [128, HT * TOK], F32),
            ("dbg_h2_1", [128, HT * TOK], BF16),
        ]:
            dbg[nm] = nc.dram_tensor(nm, shp, dt, kind="ExternalOutput")

    with tile.TileContext(nc) as tc:
        with (
            tc.tile_pool(name="const", bufs=1) as pconst,
            tc.tile_pool(name="resid", bufs=1) as presid,
            tc.tile_pool(name="norm", bufs=1) as pnorm,
            tc.tile_pool(name="wattn", bufs=2) as pwa,
            tc.tile_pool(name="wo", bufs=1) as pwo,
            tc.tile_pool(name="wmlp", bufs=2) as pwm,
            tc.tile_pool(name="attn", bufs=1) as pattn,
            tc.tile_pool(name="exp", bufs=3) as pexp,
            tc.tile_pool(name="small", bufs=2) as psmall,
            tc.tile_pool(name="act", bufs=2) as pact,
            tc.tile_pool(name="dram", bufs=2, space="DRAM") as pdram,
        ):
            # ---------------- constants ----------------
            identf = pconst.tile([128, 128], F32, tag="identf")
            make_identity(nc, identf[:])
            ident = pconst.tile([128, 128], BF16, tag="ident")
            nc.vector.tensor_copy(ident[:], identf[:])
            onesf = pconst.tile([128, 128], F32, tag="onesf")
            nc.vector.memset(onesf[:], 1.0)
            onesr = pconst.tile([128, 128], F32R, tag="onesr")
            nc.vector.tensor_copy(onesr[:], onesf[:])
            onesb = pconst.tile([128, 1], BF16, tag="onesb")
            nc.vector.tensor_copy(onesb[:], onesf[:, 0:1])
            epst = pconst.tile([128, 1], F32, tag="eps")
            nc.gpsimd.memset(epst[:], EPS)
            cosk_sb = pconst.tile([64, TOK], BF16, tag="cosk")
            sink_sb = pconst.tile([64, TOK], BF16, tag="sink")
            cosq_sb = pconst.tile([64, 8 * TOK], BF16, tag="cosq")
            sinq_sb = pconst.tile([64, 8 * TOK], BF16, tag="sinq")
            sel_sb = pconst.tile([16, MT * 128], F32, tag="sel")

            # warmup: absorb first-collective setup cost during initial DMAs
            wrm_i = pdram.tile([128, 16], BF16, tag="warm_i")
            wrm_o = pdram.tile([NCORES * 128, 16], BF16, tag="warm_o",
                               addr_space="Shared")
            nc.gpsimd.collective_compute(
                "AllGather", mybir.AluOpType.bypass, replica_groups=RG,
                ins=[wrm_i[:].opt()], outs=[wrm_o[:].opt()],
            )

            # residual (feature-major: tile ht at cols ht*TOK)
            x_sb = presid.tile([128, HT * TOK], F32, tag="x")
            nc.sync.dma_start(x_sb[:], x0[:])
            nc.sync.dma_start(cosk_sb[:], cosk[:])
            nc.sync.dma_start(sink_sb[:], sink[:])
            nc.sync.dma_start(cosq_sb[:], cosq[:])
            nc.sync.dma_start(sinq_sb[:], sinq[:])
            nc.sync.dma_start(sel_sb[:], seld[:])

            def load_attn_weights(l):
                wkv_sb = pwa.tile([128, HT * 128], BF16, tag="wkv")
                wq_sb = pwa.tile([128, MT * HT * 128], BF16, tag="wq")
                nc.sync.dma_start(wkv_sb[:], W[l]["wkv"][:])
                nc.sync.dma_start(wq_sb[:], W[l]["wq"][:])
                return wkv_sb, wq_sb

            def load_mlp_chunk(l, fc):
                wg_sb = pwm.tile([128, FM * HT * 128], BF16, tag="wg")
                wu_sb = pwm.tile([128, FM * HT * 128], BF16, tag="wu")
                wd_sb = pwm.tile([128, HT * FM * 128], BF16, tag="wd")
                nc.sync.dma_start(wg_sb[:], W[l][f"wg{fc}"][:])
                nc.sync.dma_start(wu_sb[:], W[l][f"wu{fc}"][:])
                nc.sync.dma_start(wd_sb[:], W[l][f"wd{fc}"][:])
                return wg_sb, wu_sb, wd_sb

            # initial prefetch: layer-0 attn weights + first two MLP chunks
            aw = load_attn_weights(0)
            mlp_w = [load_mlp_chunk(0, 0), load_mlp_chunk(0, 1)]

            x2 = pnorm.tile([128, HT * TOK], F32R, tag="x2")
            h_sb = pnorm.tile([128, HT * TOK], BF16, tag="h")

            def rmsnorm(tag):
                """x_sb -> h_sb (normalized, bf16)."""
                with tc.tile_pool(name=f"ps_n_{tag}", bufs=1,
                                  space="PSUM") as pps:
                    nc.scalar.square(x2[:], x_sb[:])
                    ssq = pps.tile([1, TOK], F32, tag="ssq")
                    for ht in range(HT):
                        nc.tensor.matmul(
                            ssq[:], onesr[:, 0:1],
                            x2[:, ht * TOK:(ht + 1) * TOK],
                            start=(ht == 0), stop=(ht == HT - 1),
                        )
                    sstd = psmall.tile([1, TOK], F32, tag="sstd")
                    nc.scalar.activation(
                        sstd[:], ssq[:], mybir.ActivationFunctionType.Sqrt,
                        bias=epst[0:1, :], scale=1.0 / H,
                    )
                    rinv = psmall.tile([1, TOK], F32R, tag="rinv")
                    with nc.allow_low_precision(reason="f32r is fp32 bits"):
                        nc.vector.reciprocal(rinv[:], sstd[:])
                    rb = pps.tile([128, TOK], F32, tag="rb")
                    nc.tensor.matmul(rb[:], onesr[0:1, :], rinv[:],
                                     start=True, stop=True)
                    for ht in range(HT):
                        hs = slice(ht * TOK, (ht + 1) * TOK)
                        nc.vector.tensor_tensor(
                            h_sb[:, hs], x_sb[:, hs], rb[:], op=MUL)

            q_hb = pattn.tile([64, HQ * TOK], BF16, tag="q_hb")
            rotq = pattn.tile([64, 8 * TOK], BF16, tag="rotq")
            kTt = pattn.tile([64, S], BF16, tag="kTt")
            v_tok = pattn.tile([128, KT * 65], BF16, tag="v_tok")
            for kt in range(KT):
                nc.vector.tensor_copy(
                    v_tok[:, kt * 65 + 64:kt * 65 + 65], onesb[:, 0:1])
            oT_raw = pattn.tile([128, HT * TOK], BF16, tag="oT_raw")
            oT = pattn.tile([128, HT * TOK], BF16, tag="oT")
            den0 = pattn.tile([1, HQ * TOK], F32, tag="den0")
            rec16 = pattn.tile([16, TOK], F32, tag="rec16")

            for l in range(L):
                wkv_sb, wq_sb = aw
                wo_sb = pwo.tile([128, HT * HT * 128], BF16, tag="wo")
                nc.sync.dma_start(wo_sb[:], W[l]["wo"][:])

                # ---------------- norm1 + kv proj + AG ----------------
                rmsnorm(f"a{l}")
                if DEBUG:
                    nc.sync.dma_start(
                        dbg["dbg_h" if l == 0 else "dbg_h_1"][:], h_sb[:])
                ag_in = pdram.tile([128, 256], BF16, tag="ag_in")
                with tc.tile_pool(name=f"ps_kv{l}", bufs=1,
                                  space="PSUM") as pps:
                    pkv = pps.tile([128, TOK], F32, tag="pkv")
                    for kt in range(HT):
                        nc.tensor.matmul(
                            pkv[:], wkv_sb[:, kt * 128:(kt + 1) * 128],
                            h_sb[:, kt * TOK:(kt + 1) * TOK],
                            start=(kt == 0), stop=(kt == HT - 1),
                        )
                    # rope k (feature-major, local tokens)
                    kn = psmall.tile([64, TOK], BF16, tag="kn")
                    rotk = psmall.tile([64, TOK], BF16, tag="rotk")
                    nc.vector.tensor_copy(rotk[0:32, :], pkv[32:64, :])
                    nc.vector.tensor_copy(rotk[32:64, :], pkv[0:32, :])
                    nc.vector.tensor_tensor(
                        rotk[:], rotk[:], sink_sb[:], op=MUL)
                    nc.vector.tensor_tensor(
                        kn[:], pkv[0:64, :], cosk_sb[:], op=MUL)
                    nc.vector.tensor_tensor(kn[:], kn[:], rotk[:], op=ADD)
                    # v -> token-major
                    vloc = psmall.tile([64, TOK], BF16, tag="vloc")
                    nc.vector.tensor_copy(vloc[:], pkv[64:128, :])
                    vtt = psmall.tile([128, 128], BF16, tag="vtt")
                    for j in range(2):
                        pvt = pps.tile([128, 64], BF16, tag="pvt", bufs=2)
                        nc.tensor.transpose(
                            pvt[:], vloc[:, j * 128:(j + 1) * 128],
                            ident[0:64, 0:64])
                        nc.vector.tensor_copy(
                            vtt[:, j * 64:(j + 1) * 64], pvt[:])
                    # plain-slice writes (rearranged write APs can miss
                    # dependency tracking and race the collective)
                    nc.sync.dma_start(ag_in[0:64, :], kn[:])
                    nc.sync.dma_start(ag_in[64:128, :], vtt[:])
                ag_out = pdram.tile([NCORES * 128, 256], BF16,
                                    tag="ag_out", addr_space="Shared")
                nc.gpsimd.collective_compute(
                    "AllGather", mybir.AluOpType.bypass, replica_groups=RG,
                    ins=[ag_in[:].opt()], outs=[ag_out[:].opt()],
                )

                # ---------------- q proj + rope (overlaps AG) -------------
                with tc.tile_pool(name=f"ps_q{l}", bufs=2,
                                  space="PSUM") as pps:
                    for m in range(MT):
                        pq = pps.tile([128, TOK], F32, tag="pq")
                        for kt in range(HT):
                            nc.tensor.matmul(
                                pq[:],
                                wq_sb[:, (m * HT + kt) * 128:
                                      (m * HT + kt + 1) * 128],
                                h_sb[:, kt * TOK:(kt + 1) * TOK],
                                start=(kt == 0), stop=(kt == HT - 1),
                            )
                        nc.vector.tensor_copy(
                            q_hb[:, (2 * m) * TOK:(2 * m + 1) * TOK],
                            pq[0:64, :])
                        nc.vector.tensor_copy(
                            q_hb[:, (2 * m + 1) * TOK:(2 * m + 2) * TOK],
                            pq[64:128, :])
                        if m % 4 == 3:
                            # rope this 8-head half with wide ops
                            cs = slice((m - 3) * 2 * TOK, (m + 1) * 2 * TOK)
                            W8 = slice(0, 8 * TOK)
                            nc.vector.tensor_copy(
                                rotq[0:32, W8], q_hb[32:64, cs])
                            nc.vector.tensor_copy(
                                rotq[32:64, W8], q_hb[0:32, cs])
                            nc.vector.tensor_tensor(
                                rotq[:, W8], rotq[:, W8], sinq_sb[:, W8],
                                op=MUL)
                            nc.vector.tensor_tensor(
                                q_hb[:, cs], q_hb[:, cs], cosq_sb[:, W8],
                                op=MUL)
                            nc.vector.tensor_tensor(
                                q_hb[:, cs], q_hb[:, cs], rotq[:, W8], op=ADD)

                # load gathered k/v (k feature-major, v token-major tiles)
                for c in range(NCORES):
                    nc.sync.dma_start(
                        kTt[:, c * TOK:(c + 1) * TOK],
                        ag_out[c * 128:c * 128 + 64, :])
                for kt in range(KT):
                    c, a = kt // 2, kt % 2
                    nc.sync.dma_start(
                        v_tok[:, kt * 65:kt * 65 + 64],
                        ag_out[c * 128 + 64:(c + 1) * 128, :]
                        .rearrange("a (b j d) -> (a b) j d", b=2, j=2)
                        [:, a:a + 1, :])

                if DEBUG:
                    sfx = "" if l == 0 else "_1"
                    nc.sync.dma_start(dbg["dbg_q" + sfx][:], q_hb[:])
                    nc.sync.dma_start(dbg["dbg_kT" + sfx][:], kTt[:])
                    nc.sync.dma_start(dbg["dbg_vt" + sfx][:], v_tok[:])

                # prefetch next-layer attn weights early (during attention)
                if l + 1 < L:
                    aw = load_attn_weights(l + 1)

                # ---------------- attention core ----------------
                with (
                    tc.tile_pool(name=f"ps_sc{l}", bufs=2,
                                 space="PSUM") as ppsc,
                    tc.tile_pool(name=f"ps_av{l}", bufs=2,
                                 space="PSUM") as ppav,
                ):
                    for g in range(4):      # 4 heads per group
                        q0 = g * 4 * TOK
                        pav = ppav.tile([65, 4 * TOK], F32, tag="pav")
                        psc_l = []
                        for kt in range(KT):
                            psc = ppsc.tile([128, 4 * TOK], F32, tag="psc")
                            for j in range(2):
                                nc.tensor.matmul(
                                    psc[:, j * 512:(j + 1) * 512],
                                    kTt[:, kt * 128:(kt + 1) * 128],
                                    q_hb[:, q0 + j * 512:q0 + (j + 1) * 512],
                                    start=True, stop=True,
                                )
                            et = pexp.tile([128, 4 * TOK], BF16, tag="et")
                            nc.scalar.activation(
                                et[:], psc[:],
                                mybir.ActivationFunctionType.Exp)
                            psc_l.append(et)
                            if kt > 0:
                                etp = psc_l[kt - 1]
                                for j in range(2):
                                    nc.tensor.matmul(
                                        pav[:, j * 512:(j + 1) * 512],
                                        v_tok[:, (kt - 1) * 65:kt * 65],
                                        etp[:, j * 512:(j + 1) * 512],
                                        start=(kt - 1 == 0), stop=False,
                                        skip_group_check=True,
                                    )
                        etp = psc_l[KT - 1]
                        for j in range(2):
                            nc.tensor.matmul(
                                pav[:, j * 512:(j + 1) * 512],
                                v_tok[:, (KT - 1) * 65:KT * 65],
                                etp[:, j * 512:(j + 1) * 512],
                                start=False, stop=True,
                                skip_group_check=True,
                            )
                        # denominator reciprocals + numerator extraction
                        with nc.allow_low_precision(
                                reason="full fp32 reciprocal"):
                            nc.vector.reciprocal(
                                den0[0:1, g * 4 * TOK:(g + 1) * 4 * TOK],
                                pav[64:65, :])
                        for j in range(4):
                            h = 4 * g + j
                            nc.vector.tensor_copy(
                                oT_raw[(h % 2) * 64:(h % 2) * 64 + 64,
                                       (h // 2) * TOK:(h // 2 + 1) * TOK],
                                pav[0:64, j * TOK:(j + 1) * TOK])

                # ---------------- normalize + o proj ----------------
                # spread the 16 head-denominator rows across partitions
                # (SBUF->SBUF DMA has no partition-alignment restriction)
                nc.sync.dma_start(rec16[:], den0[0:1, :])
                if DEBUG:
                    nc.sync.dma_start(
                        dbg["dbg_den" if l == 0 else "dbg_den_1"][:], den0[:])
                with tc.tile_pool(name=f"ps_o{l}", bufs=2,
                                  space="PSUM") as ppo:
                    for m in range(MT):
                        prb = ppo.tile([128, TOK], F32, tag="prb")
                        nc.tensor.matmul(
                            prb[:], sel_sb[:, m * 128:(m + 1) * 128],
                            rec16[:], start=True, stop=True)
                        ms = slice(m * TOK, (m + 1) * TOK)
                        nc.vector.tensor_tensor(
                            oT[:, ms], oT_raw[:, ms], prb[:], op=MUL)
                    if DEBUG:
                        nc.sync.dma_start(
                            dbg["dbg_oT" if l == 0 else "dbg_oT_1"][:], oT[:])
                    for m in range(HT):
                        po = ppo.tile([128, TOK], F32, tag="po")
                        for kt in range(HT):
                            nc.tensor.matmul(
                                po[:],
                                wo_sb[:, (m * HT + kt) * 128:
                                      (m * HT + kt + 1) * 128],
                                oT[:, kt * TOK:(kt + 1) * TOK],
                                start=(kt == 0), stop=(kt == HT - 1),
                            )
                        ms = slice(m * TOK, (m + 1) * TOK)
                        nc.vector.tensor_tensor(
                            x_sb[:, ms], x_sb[:, ms], po[:], op=ADD)

                # ---------------- norm2 + MLP ----------------
                if DEBUG:
                    nc.sync.dma_start(
                        dbg["dbg_x1" if l == 0 else "dbg_x1_1"][:], x_sb[:])
                rmsnorm(f"m{l}")
                if DEBUG:
                    nc.sync.dma_start(
                        dbg["dbg_h2" if l == 0 else "dbg_h2_1"][:], h_sb[:])
                with (
                    tc.tile_pool(name=f"ps_d{l}", bufs=1, space="PSUM") as ppd,
                    tc.tile_pool(name=f"ps_gu{l}", bufs=2,
                                 space="PSUM") as ppg,
                ):
                    pd = ppd.tile([128, HT * TOK], F32, tag="pd")
                    # 8 independent 1KB accumulation regions share PSUM
                    # banks, and matmul start=True zeroes the whole bank:
                    # init by memset and accumulate with start=False.
                    nc.vector.memset(pd[:], 0.0)
                    NSTEP = FC * FM         # 32 global (fc, fm) steps
                    acts = {}

                    def emit_down(i):
                        fc, fk = i // FM, i % FM
                        wd_sb = mlp_w[fc % 2][2]
                        a = acts.pop(i)
                        for m in range(HT):
                            nc.tensor.matmul(
                                pd[:, m * TOK:(m + 1) * TOK],
                                wd_sb[:, (m * FM + fk) * 128:
                                      (m * FM + fk + 1) * 128],
                                a[:],
                                start=False, stop=(i == NSTEP - 1),
                                skip_group_check=True,
                            )

                    for i in range(NSTEP):
                        fc, fm = i // FM, i % FM
                        wg_sb, wu_sb, _ = mlp_w[fc % 2]
                        if i > 0:
                            # down for the previous step BEFORE the prefetch
                            # below recycles its weight buffer
                            emit_down(i - 1)
                        if fm == 0 and 1 <= fc < FC - 1:
                            # prefetch chunk fc+1 (chunks 0/1 preloaded)
                            mlp_w[(fc + 1) % 2] = load_mlp_chunk(l, fc + 1)
                        pg = ppg.tile([128, TOK], F32, tag="pg")
                        pu = ppg.tile([128, TOK], F32, tag="pu")
                        for kt in range(HT):
                            rhs = h_sb[:, kt * TOK:(kt + 1) * TOK]
                            nc.tensor.matmul(
                                pg[:],
                                wg_sb[:, (fm * HT + kt) * 128:
                                      (fm * HT + kt + 1) * 128],
                                rhs, start=(kt == 0), stop=(kt == HT - 1))
                            nc.tensor.matmul(
                                pu[:],
                                wu_sb[:, (fm * HT + kt) * 128:
                                      (fm * HT + kt + 1) * 128],
                                rhs, start=(kt == 0), stop=(kt == HT - 1))
                        sg = pact.tile([128, TOK], BF16, tag="sg")
                        nc.scalar.activation(
                            sg[:], pg[:], mybir.ActivationFunctionType.Silu)
                        a = pact.tile([128, TOK], BF16, tag="a", bufs=3)
                        nc.vector.tensor_tensor(a[:], sg[:], pu[:], op=MUL)
                        acts[i] = a
                    emit_down(NSTEP - 1)
                    if l + 1 < L:
                        # next layer's first two chunks (fetched during its
                        # attention phase; WAR-safe: emitted after all reads)
                        mlp_w = [load_mlp_chunk(l + 1, 0),
                                 load_mlp_chunk(l + 1, 1)]
                    for m in range(HT):
                        ms = slice(m * TOK, (m + 1) * TOK)
                        nc.vector.tensor_tensor(
                            x_sb[:, ms], x_sb[:, ms], pd[:, ms], op=ADD)
                if DEBUG and l == 0:
                    nc.sync.dma_start(dbg["dbg_x2"][:], x_sb[:])

            nc.sync.dma_start(xout[:], x_sb[:])
    return nc


def _get_nc():
    global _CACHED_NC
    if _CACHED_NC is None:
        _CACHED_NC = _build_nc()
    return _CACHED_NC


def _sel_matrix():
    s = np.zeros((16, MT * 128), np.float32)
    for m in range(MT):
        for j in range(2):
            s[2 * m + j, m * 128 + j * 64:m * 128 + (j + 1) * 64] = 1.0
    return s


def _host_prep(inputs):
    """Fold ln/scale into weights, build lhsT-layout replicated weight arrays
    (shared across cores), embed gather + per-core residual slices, rope
    tables.  Returns in_maps (list of dicts, one per core)."""
    ids = np.asarray(inputs["input_ids"])[0]          # [S] int32
    embed = np.asarray(inputs["embed"], np.float32)   # [V, H]
    x = embed[ids]                                    # [S, H]

    inv = 1.0 / (10000.0 ** (np.arange(0, D, 2, dtype=np.float32) / D))
    freqs = np.arange(S, dtype=np.float32)[:, None] * inv[None, :]  # [S,32]
    cosT = np.cos(freqs).T.astype(np.float32)   # [32, S]
    sinT = np.sin(freqs).T.astype(np.float32)
    cosF = np.tile(cosT, (2, 1))                            # [64, S]
    sinF = np.concatenate([-sinT, sinT], 0)                 # [64, S]

    def bf(a):
        return np.ascontiguousarray(a).astype(BF)

    scale = np.float32(1.0 / np.sqrt(D))
    shared = {}
    for l in range(L):
        ln1 = np.asarray(inputs["ln1"], np.float32)[l]
        ln2 = np.asarray(inputs["ln2"], np.float32)[l]
        wq = np.asarray(inputs["Wq"], np.float32)[l] * ln1[None, :] * scale
        wk = np.asarray(inputs["Wk"], np.float32)[l] * ln1[None, :]
        wv = np.asarray(inputs["Wv"], np.float32)[l] * ln1[None, :]
        wo = np.asarray(inputs["Wo"], np.float32)[l]
        wg = np.asarray(inputs["Wg"], np.float32)[l] * ln2[None, :]
        wu = np.asarray(inputs["Wu"], np.float32)[l] * ln2[None, :]
        wd = np.asarray(inputs["Wd"], np.float32)[l]

        # wkv: [128, (kt)*128]: rows=in-dims of kt, cols=[64 k | 64 v]
        kv = np.concatenate([wk, wv], 0).T                  # [H, 128]
        shared[f"wkv{l}"] = bf(
            kv.reshape(HT, 128, 128).transpose(1, 0, 2).reshape(128, -1))
        # wq: blocks (m*HT+kt): T[kt*128+r, m*128+c]
        T = wq.T                                            # [in, out]
        shared[f"wq{l}"] = bf(
            T.reshape(HT, 128, MT, 128).transpose(1, 2, 0, 3)
            .reshape(128, -1))
        # wo: blocks (m*HT+kt): rows=o-in dims kt, cols=H dims m
        T = wo.T                                            # [o-in, H]
        shared[f"wo{l}"] = bf(
            T.reshape(HT, 128, HT, 128).transpose(1, 2, 0, 3)
            .reshape(128, -1))
        # wg/wu chunks: blocks (fm*HT+kt): rows=in-dims kt, cols=FF dims
        for name, wt in (("wg", wg), ("wu", wu)):
            T = wt.T                                        # [H, FF]
            Tb = T.reshape(HT, 128, FC, FM, 128)
            for fc in range(FC):
                shared[f"{name}{l}_{fc}"] = bf(
                    Tb[:, :, fc].transpose(1, 2, 0, 3).reshape(128, -1))
        # wd chunks: blocks (m*FM+fk): rows=FF dims (fc,fk), cols=H dims m
        T = wd.T                                            # [FF, H]
        Tb = T.reshape(FC, FM, 128, HT, 128)
        for fc in range(FC):
            # (fk, r, m, c) -> (r, m, fk, c): block (m*FM+fk), partition r
            shared[f"wd{l}_{fc}"] = bf(
                Tb[fc].transpose(1, 2, 0, 3).reshape(128, -1))

    in_maps = []
    for c in range(NCORES):
        sl = slice(c * TOK, (c + 1) * TOK)
        xT = np.ascontiguousarray(x[sl].T)                  # [H, TOK]
        m = {
            "x0": np.ascontiguousarray(
                xT.reshape(HT, 128, TOK).transpose(1, 0, 2)
                .reshape(128, -1)),
            "cosk": bf(cosF[:, sl]),
            "sink": bf(sinF[:, sl]),
            "cosq": bf(np.tile(cosF[:, sl], (1, 8))),
            "sinq": bf(np.tile(sinF[:, sl], (1, 8))),
            "sel": _sel_matrix(),
        }
        m.update(shared)
        in_maps.append(m)
    return in_maps


def kernel(**inputs) -> np.ndarray:
    nc = _get_nc()
    in_maps = _host_prep(inputs)
    res = bass_utils.run_bass_kernel_spmd(
        nc, in_maps, core_ids=list(range(NCORES))
    )
    out = np.empty((1, S, H), np.float32)
    for c in range(NCORES):
        xo = res.results[c]["xout"]            # [128, HT*TOK]
        out[0, c * TOK:(c + 1) * TOK, :] = (
            xo.reshape(128, HT, TOK).transpose(1, 0, 2)
            .reshape(H, TOK).T)
    return out


# revision 3
# speedup vs baseline: 1.0002x; 1.0002x over previous
"""Trainium2 Bass kernel for nn_CustomLlamaModel (2-layer MQA llama, B=1 S=2048
H=1024 HQ=16 HKV=1 FF=4096), fully token-sharded data-parallel over 8 cores.

Each core owns 256 tokens end-to-end; all weights are replicated (streamed
from HBM, overlapped with compute).  The only per-layer collective is a tiny
k/v AllGather (64KB in / 512KB out per core): k,v are computed from the local
token shard, k roped locally, v transposed to token-major, gathered, and
reassembled (k feature-major [64,2048] lhsT for scoresT, v token-major
[128,16x65] with a folded ones column so the softmax denominator rides the
attn@v matmul).  No 4MB AllGathers / ReduceScatters, no collective dead
windows.  Attention runs per 4-head group with N=512 matmuls, software-
pipelined so the PE never waits on the Exp activation.  MLP streams FF in
512-wide chunks, gate/up one step ahead of the down accumulation.  ln1/ln2
and the 1/sqrt(D) scale are folded into weights host-side; the embedding
gather runs host-side.
"""
import os
import sys

sys.path.insert(0, "/opt/trn_rl_repo")

import ml_dtypes
import numpy as np
import orjson

import concourse.bass as bass
import concourse.mybir as mybir
import concourse.tile as tile
from concourse import bass_utils
from concourse.masks import make_identity

# ---------------------------------------------------------------------------
# Walrus in this container supports only ONE sync-wait per instruction, but
# Tile's scheduler emits multi-wait instructions.  Post-process the BIR JSON:
# split each multi-wait instruction into single-wait NoOps (same engine,
# program-order before the original).
# ---------------------------------------------------------------------------
_orig_to_json_bytes = bass.Bass.to_json_bytes
_MW = [0]


def _split_multiwait(d):
    changed = False

    def fix_block(bb):
        nonlocal changed
        insts = bb.get("instructions")
        if not insts:
            return
        out = []
        for ins in insts:
            si = ins.get("sync_info")
            if si:
                ow = si.get("on_wait") or []
                if len(ow) > 1:
                    changed = True
                    for w in ow[:-1]:
                        _MW[0] += 1
                        out.append({
                            "debug": ins.get("debug", 0),
                            "engine": ins["engine"],
                            "ins": [],
                            "outs": [],
                            "name": f"{ins['name']}-mw{_MW[0]}",
                            "opcode": "NoOp",
                            "sync_info": {"on_update": [], "on_wait": [w]},
                        })
                    si["on_wait"] = [ow[-1]]
            out.append(ins)
        bb["instructions"] = out

    def rec(o):
        if isinstance(o, dict):
            if isinstance(o.get("instructions"), list):
                fix_block(o)
            for v in o.values():
                rec(v)
        elif isinstance(o, list):
            for v in o:
                rec(v)

    for fn in d.get("functions", []):
        rec(fn)
    return changed


def _patched_to_json_bytes(self):
    raw = _orig_to_json_bytes(self)
    d = orjson.loads(raw)
    if _split_multiwait(d):
        return orjson.dumps(d)
    return raw


bass.Bass.to_json_bytes = _patched_to_json_bytes

# ---------------------------------------------------------------------------
# Model / sharding constants
# ---------------------------------------------------------------------------
S, H, D, HQ, FF, L, V = 2048, 1024, 64, 16, 4096, 2, 32000
EPS = 1e-6
NCORES = 8
TOK = S // NCORES       # tokens per core (256)
HT = H // 128           # 8 hidden feature tiles
KT = S // 128           # 16 key-token tiles
MT = HQ // 2            # 8 q m-tiles (2 heads each)
FC = 8                  # FF chunks
FCW = FF // FC          # 512
FM = FCW // 128         # 4 sub-tiles per chunk
F32 = mybir.dt.float32
F32R = mybir.dt.float32r
BF16 = mybir.dt.bfloat16
MUL = mybir.AluOpType.mult
ADD = mybir.AluOpType.add
RG = [list(range(NCORES))]
BF = ml_dtypes.bfloat16

DEBUG = os.environ.get("KDP_DEBUG") == "1"
_CACHED_NC = None


def _build_nc():
    nc = bass.Bass()
    x0 = nc.dram_tensor("x0", [128, HT * TOK], F32, kind="ExternalInput")
    cosk = nc.dram_tensor("cosk", [64, TOK], BF16, kind="ExternalInput")
    sink = nc.dram_tensor("sink", [64, TOK], BF16, kind="ExternalInput")
    cosq = nc.dram_tensor("cosq", [64, 8 * TOK], BF16, kind="ExternalInput")
    sinq = nc.dram_tensor("sinq", [64, 8 * TOK], BF16, kind="ExternalInput")
    seld = nc.dram_tensor("sel", [16, MT * 128], F32, kind="ExternalInput")
    W = []
    for l in range(L):
        w = {
            "wkv": nc.dram_tensor(f"wkv{l}", [128, HT * 128], BF16,
                                  kind="ExternalInput"),
            "wq": nc.dram_tensor(f"wq{l}", [128, MT * HT * 128], BF16,
                                 kind="ExternalInput"),
            "wo": nc.dram_tensor(f"wo{l}", [128, HT * HT * 128], BF16,
                                 kind="ExternalInput"),
        }
        for fc in range(FC):
            w[f"wg{fc}"] = nc.dram_tensor(f"wg{l}_{fc}", [128, FM * HT * 128],
                                          BF16, kind="ExternalInput")
            w[f"wu{fc}"] = nc.dram_tensor(f"wu{l}_{fc}", [128, FM * HT * 128],
                                          BF16, kind="ExternalInput")
            w[f"wd{fc}"] = nc.dram_tensor(f"wd{l}_{fc}", [128, HT * FM * 128],
                                          BF16, kind="ExternalInput")
        W.append(w)
    xout = nc.dram_tensor("xout", [128, HT * TOK], F32, kind="ExternalOutput")
    dbg = {}
    if DEBUG:
        for nm, shp, dt in [
            ("dbg_h", [128, HT * TOK], BF16),
            ("dbg_q", [64, HQ * TOK], BF16),
            ("dbg_kT", [64, S], BF16),
            ("dbg_vt", [128, KT * 65], BF16),
            ("dbg_den", [1, HQ * TOK], F32),
            ("dbg_oT", [128, HT * TOK], BF16),
            ("dbg_x1", [128, HT * TOK], F32),
            ("dbg_h2", [128, HT * TOK], BF16),
            ("dbg_x2", [128, HT * TOK], F32),
            ("dbg_h_1", [128, HT * TOK], BF16),
            ("dbg_q_1", [64, HQ * TOK], BF16),
            ("dbg_kT_1", [64, S], BF16),
            ("dbg_vt_1", [128, KT * 65], BF16),
            ("dbg_den_1", [1, HQ * TOK], F32),
            ("dbg_oT_1", [128, HT * TOK], BF16),
            ("dbg_x1_1", [128, HT * TOK], F32),
            ("dbg_h2_1", [128, HT * TOK], BF16),
        ]:
            dbg[nm] = nc.dram_tensor(nm, shp, dt, kind="ExternalOutput")

    with tile.TileContext(nc) as tc:
        with (
            tc.tile_pool(name="const", bufs=1) as pconst,
            tc.tile_pool(name="resid", bufs=1) as presid,
            tc.tile_pool(name="norm", bufs=1) as pnorm,
            tc.tile_pool(name="wattn", bufs=2) as pwa,
            tc.tile_pool(name="wo", bufs=1) as pwo,
            tc.tile_pool(name="wmlp", bufs=2) as pwm,
            tc.tile_pool(name="attn", bufs=1) as pattn,
            tc.tile_pool(name="exp", bufs=3) as pexp,
            tc.tile_pool(name="small", bufs=2) as psmall,
            tc.tile_pool(name="act", bufs=2) as pact,
            tc.tile_pool(name="dram", bufs=2, space="DRAM") as pdram,
        ):
            # ---------------- constants ----------------
            identf = pconst.tile([128, 128], F32, tag="identf")
            make_identity(nc, identf[:])
            ident = pconst.tile([128, 128], BF16, tag="ident")
            nc.vector.tensor_copy(ident[:], identf[:])
            onesf = pconst.tile([128, 128], F32, tag="onesf")
            nc.vector.memset(onesf[:], 1.0)
            onesr = pconst.tile([128, 128], F32R, tag="onesr")
            nc.vector.tensor_copy(onesr[:], onesf[:])
            onesb = pconst.tile([128, 1], BF16, tag="onesb")
            nc.vector.tensor_copy(onesb[:], onesf[:, 0:1])
            epst = pconst.tile([128, 1], F32, tag="eps")
            nc.gpsimd.memset(epst[:], EPS)
            cosk_sb = pconst.tile([64, TOK], BF16, tag="cosk")
            sink_sb = pconst.tile([64, TOK], BF16, tag="sink")
            cosq_sb = pconst.tile([64, 8 * TOK], BF16, tag="cosq")
            sinq_sb = pconst.tile([64, 8 * TOK], BF16, tag="sinq")
            sel_sb = pconst.tile([16, MT * 128], F32, tag="sel")

            # warmup: absorb first-collective setup cost during initial DMAs
            wrm_i = pdram.tile([128, 16], BF16, tag="warm_i")
            wrm_o = pdram.tile([NCORES * 128, 16], BF16, tag="warm_o",
                               addr_space="Shared")
            nc.gpsimd.collective_compute(
                "AllGather", mybir.AluOpType.bypass, replica_groups=RG,
                ins=[wrm_i[:].opt()], outs=[wrm_o[:].opt()],
            )

            # residual (feature-major: tile ht at cols ht*TOK)
            x_sb = presid.tile([128, HT * TOK], F32, tag="x")
            nc.sync.dma_start(x_sb[:], x0[:])
            nc.sync.dma_start(cosk_sb[:], cosk[:])
            nc.sync.dma_start(sink_sb[:], sink[:])
            nc.sync.dma_start(cosq_sb[:], cosq[:])
            nc.sync.dma_start(sinq_sb[:], sinq[:])
            nc.sync.dma_start(sel_sb[:], seld[:])

            def load_attn_weights(l):
                wkv_sb = pwa.tile([128, HT * 128], BF16, tag="wkv")
                wq_sb = pwa.tile([128, MT * HT * 128], BF16, tag="wq")
                nc.sync.dma_start(wkv_sb[:], W[l]["wkv"][:])
                nc.sync.dma_start(wq_sb[:], W[l]["wq"][:])
                return wkv_sb, wq_sb

            def load_mlp_chunk(l, fc):
                wg_sb = pwm.tile([128, FM * HT * 128], BF16, tag="wg")
                wu_sb = pwm.tile([128, FM * HT * 128], BF16, tag="wu")
                wd_sb = pwm.tile([128, HT * FM * 128], BF16, tag="wd")
                nc.sync.dma_start(wg_sb[:], W[l][f"wg{fc}"][:])
                nc.sync.dma_start(wu_sb[:], W[l][f"wu{fc}"][:])
                nc.sync.dma_start(wd_sb[:], W[l][f"wd{fc}"][:])
                return wg_sb, wu_sb, wd_sb

            # initial prefetch: layer-0 attn weights + first two MLP chunks
            aw = load_attn_weights(0)
            mlp_w = [load_mlp_chunk(0, 0), load_mlp_chunk(0, 1)]

            x2 = pnorm.tile([128, HT * TOK], F32R, tag="x2")
            h_sb = pnorm.tile([128, HT * TOK], BF16, tag="h")

            def rmsnorm(tag):
                """x_sb -> h_sb (normalized, bf16)."""
                with tc.tile_pool(name=f"ps_n_{tag}", bufs=1,
                                  space="PSUM") as pps:
                    nc.scalar.square(x2[:], x_sb[:])
                    ssq = pps.tile([1, TOK], F32, tag="ssq")
                    for ht in range(HT):
                        nc.tensor.matmul(
                            ssq[:], onesr[:, 0:1],
                            x2[:, ht * TOK:(ht + 1) * TOK],
                            start=(ht == 0), stop=(ht == HT - 1),
                        )
                    sstd = psmall.tile([1, TOK], F32, tag="sstd")
                    nc.scalar.activation(
                        sstd[:], ssq[:], mybir.ActivationFunctionType.Sqrt,
                        bias=epst[0:1, :], scale=1.0 / H,
                    )
                    rinv = psmall.tile([1, TOK], F32R, tag="rinv")
                    with nc.allow_low_precision(reason="f32r is fp32 bits"):
                        nc.vector.reciprocal(rinv[:], sstd[:])
                    rb = pps.tile([128, TOK], F32, tag="rb")
                    nc.tensor.matmul(rb[:], onesr[0:1, :], rinv[:],
                                     start=True, stop=True)
                    for ht in range(HT):
                        hs = slice(ht * TOK, (ht + 1) * TOK)
                        nc.vector.tensor_tensor(
                            h_sb[:, hs], x_sb[:, hs], rb[:], op=MUL)

            q_hb = pattn.tile([64, HQ * TOK], BF16, tag="q_hb")
            rotq = pattn.tile([64, 8 * TOK], BF16, tag="rotq")
            kTt = pattn.tile([64, S], BF16, tag="kTt")
            v_tok = pattn.tile([128, KT * 65], BF16, tag="v_tok")
            for kt in range(KT):
                nc.vector.tensor_copy(
                    v_tok[:, kt * 65 + 64:kt * 65 + 65], onesb[:, 0:1])
            oT_raw = pattn.tile([128, HT * TOK], BF16, tag="oT_raw")
            oT = pattn.tile([128, HT * TOK], BF16, tag="oT")
            den0 = pattn.tile([1, HQ * TOK], F32, tag="den0")
            den16 = pattn.tile([16, TOK], F32, tag="den16")
            rec16 = pattn.tile([16, TOK], F32, tag="rec16")

            for l in range(L):
                wkv_sb, wq_sb = aw
                wo_sb = pwo.tile([128, HT * HT * 128], BF16, tag="wo")
                nc.sync.dma_start(wo_sb[:], W[l]["wo"][:])

                # ---------------- norm1 + kv proj + AG ----------------
                rmsnorm(f"a{l}")
                if DEBUG:
                    nc.sync.dma_start(
                        dbg["dbg_h" if l == 0 else "dbg_h_1"][:], h_sb[:])
                ag_in = pdram.tile([128, 256], BF16, tag="ag_in")
                with tc.tile_pool(name=f"ps_kv{l}", bufs=1,
                                  space="PSUM") as pps:
                    pkv = pps.tile([128, TOK], F32, tag="pkv")
                    for kt in range(HT):
                        nc.tensor.matmul(
                            pkv[:], wkv_sb[:, kt * 128:(kt + 1) * 128],
                            h_sb[:, kt * TOK:(kt + 1) * TOK],
                            start=(kt == 0), stop=(kt == HT - 1),
                        )
                    # rope k (feature-major, local tokens)
                    kn = psmall.tile([64, TOK], BF16, tag="kn")
                    rotk = psmall.tile([64, TOK], BF16, tag="rotk")
                    nc.vector.tensor_copy(rotk[0:32, :], pkv[32:64, :])
                    nc.vector.tensor_copy(rotk[32:64, :], pkv[0:32, :])
                    nc.vector.tensor_tensor(
                        rotk[:], rotk[:], sink_sb[:], op=MUL)
                    nc.vector.tensor_tensor(
                        kn[:], pkv[0:64, :], cosk_sb[:], op=MUL)
                    nc.vector.tensor_tensor(kn[:], kn[:], rotk[:], op=ADD)
                    # v -> token-major
                    vloc = psmall.tile([64, TOK], BF16, tag="vloc")
                    nc.vector.tensor_copy(vloc[:], pkv[64:128, :])
                    vtt = psmall.tile([128, 128], BF16, tag="vtt")
                    for j in range(2):
                        pvt = pps.tile([128, 64], BF16, tag="pvt", bufs=2)
                        nc.tensor.transpose(
                            pvt[:], vloc[:, j * 128:(j + 1) * 128],
                            ident[0:64, 0:64])
                        nc.vector.tensor_copy(
                            vtt[:, j * 64:(j + 1) * 64], pvt[:])
                    # plain-slice writes (rearranged write APs can miss
                    # dependency tracking and race the collective)
                    nc.sync.dma_start(ag_in[0:64, :], kn[:])
                    nc.sync.dma_start(ag_in[64:128, :], vtt[:])
                ag_out = pdram.tile([NCORES * 128, 256], BF16,
                                    tag="ag_out", addr_space="Shared")
                nc.gpsimd.collective_compute(
                    "AllGather", mybir.AluOpType.bypass, replica_groups=RG,
                    ins=[ag_in[:].opt()], outs=[ag_out[:].opt()],
                )

                # ---------------- q proj + rope (overlaps AG) -------------
                with tc.tile_pool(name=f"ps_q{l}", bufs=2,
                                  space="PSUM") as pps:
                    for m in range(MT):
                        pq = pps.tile([128, TOK], F32, tag="pq")
                        for kt in range(HT):
                            nc.tensor.matmul(
                                pq[:],
                                wq_sb[:, (m * HT + kt) * 128:
                                      (m * HT + kt + 1) * 128],
                                h_sb[:, kt * TOK:(kt + 1) * TOK],
                                start=(kt == 0), stop=(kt == HT - 1),
                            )
                        nc.vector.tensor_copy(
                            q_hb[:, (2 * m) * TOK:(2 * m + 1) * TOK],
                            pq[0:64, :])
                        nc.vector.tensor_copy(
                            q_hb[:, (2 * m + 1) * TOK:(2 * m + 2) * TOK],
                            pq[64:128, :])
                        if m % 4 == 3:
                            # rope this 8-head half with wide ops
                            cs = slice((m - 3) * 2 * TOK, (m + 1) * 2 * TOK)
                            W8 = slice(0, 8 * TOK)
                            nc.vector.tensor_copy(
                                rotq[0:32, W8], q_hb[32:64, cs])
                            nc.vector.tensor_copy(
                                rotq[32:64, W8], q_hb[0:32, cs])
                            nc.vector.tensor_tensor(
                                rotq[:, W8], rotq[:, W8], sinq_sb[:, W8],
                                op=MUL)
                            nc.vector.tensor_tensor(
                                q_hb[:, cs], q_hb[:, cs], cosq_sb[:, W8],
                                op=MUL)
                            nc.vector.tensor_tensor(
                                q_hb[:, cs], q_hb[:, cs], rotq[:, W8], op=ADD)

                # load gathered k/v (k feature-major, v token-major tiles)
                for c in range(NCORES):
                    nc.sync.dma_start(
                        kTt[:, c * TOK:(c + 1) * TOK],
                        ag_out[c * 128:c * 128 + 64, :])
                for kt in range(KT):
                    c, a = kt // 2, kt % 2
                    nc.sync.dma_start(
                        v_tok[:, kt * 65:kt * 65 + 64],
                        ag_out[c * 128 + 64:(c + 1) * 128, :]
                        .rearrange("a (b j d) -> (a b) j d", b=2, j=2)
                        [:, a:a + 1, :])

                if DEBUG:
                    sfx = "" if l == 0 else "_1"
                    nc.sync.dma_start(dbg["dbg_q" + sfx][:], q_hb[:])
                    nc.sync.dma_start(dbg["dbg_kT" + sfx][:], kTt[:])
                    nc.sync.dma_start(dbg["dbg_vt" + sfx][:], v_tok[:])

                # prefetch next-layer attn weights early (during attention)
                if l + 1 < L:
                    aw = load_attn_weights(l + 1)

                # ---------------- attention core ----------------
                with (
                    tc.tile_pool(name=f"ps_sc{l}", bufs=2,
                                 space="PSUM") as ppsc,
                    tc.tile_pool(name=f"ps_av{l}", bufs=2,
                                 space="PSUM") as ppav,
                ):
                    for g in range(4):      # 4 heads per group
                        q0 = g * 4 * TOK
                        pav = ppav.tile([65, 4 * TOK], F32, tag="pav")
                        psc_l = []
                        for kt in range(KT):
                            psc = ppsc.tile([128, 4 * TOK], F32, tag="psc")
                            for j in range(2):
                                nc.tensor.matmul(
                                    psc[:, j * 512:(j + 1) * 512],
                                    kTt[:, kt * 128:(kt + 1) * 128],
                                    q_hb[:, q0 + j * 512:q0 + (j + 1) * 512],
                                    start=True, stop=True,
                                )
                            et = pexp.tile([128, 4 * TOK], BF16, tag="et")
                            nc.scalar.activation(
                                et[:], psc[:],
                                mybir.ActivationFunctionType.Exp)
                            psc_l.append(et)
                            if kt > 0:
                                etp = psc_l[kt - 1]
                                for j in range(2):
                                    nc.tensor.matmul(
                                        pav[:, j * 512:(j + 1) * 512],
                                        v_tok[:, (kt - 1) * 65:kt * 65],
                                        etp[:, j * 512:(j + 1) * 512],
                                        start=(kt - 1 == 0), stop=False,
                                        skip_group_check=True,
                                    )
                        etp = psc_l[KT - 1]
                        for j in range(2):
                            nc.tensor.matmul(
                                pav[:, j * 512:(j + 1) * 512],
                                v_tok[:, (KT - 1) * 65:KT * 65],
                                etp[:, j * 512:(j + 1) * 512],
                                start=False, stop=True,
                                skip_group_check=True,
                            )
                        # stash denominators (reciprocal is batched later:
                        # a [1,1024] 1-partition reciprocal costs ~5us)
                        nc.vector.tensor_copy(
                            den0[0:1, g * 4 * TOK:(g + 1) * 4 * TOK],
                            pav[64:65, :])
                        for j in range(4):
                            h = 4 * g + j
                            nc.vector.tensor_copy(
                                oT_raw[(h % 2) * 64:(h % 2) * 64 + 64,
                                       (h // 2) * TOK:(h // 2 + 1) * TOK],
                                pav[0:64, j * TOK:(j + 1) * TOK])

                # ---------------- normalize + o proj ----------------
                # spread the 16 head-denominator rows across partitions
                # (SBUF->SBUF DMA has no partition-alignment restriction)
                nc.sync.dma_start(den16[:], den0[0:1, :])
                with nc.allow_low_precision(reason="full fp32 reciprocal"):
                    nc.vector.reciprocal(rec16[:], den16[:])
                if DEBUG:
                    nc.sync.dma_start(
                        dbg["dbg_den" if l == 0 else "dbg_den_1"][:], den0[:])
                with tc.tile_pool(name=f"ps_o{l}", bufs=2,
                                  space="PSUM") as ppo:
                    for m in range(MT):
                        prb = ppo.tile([128, TOK], F32, tag="prb")
                        nc.tensor.matmul(
                            prb[:], sel_sb[:, m * 128:(m + 1) * 128],
                            rec16[:], start=True, stop=True)
                        ms = slice(m * TOK, (m + 1) * TOK)
                        nc.vector.tensor_tensor(
                            oT[:, ms], oT_raw[:, ms], prb[:], op=MUL)
                    if DEBUG:
                        nc.sync.dma_start(
                            dbg["dbg_oT" if l == 0 else "dbg_oT_1"][:], oT[:])
                    for m in range(HT):
                        po = ppo.tile([128, TOK], F32, tag="po")
                        for kt in range(HT):
                            nc.tensor.matmul(
                                po[:],
                                wo_sb[:, (m * HT + kt) * 128:
                                      (m * HT + kt + 1) * 128],
                                oT[:, kt * TOK:(kt + 1) * TOK],
                                start=(kt == 0), stop=(kt == HT - 1),
                            )
                        ms = slice(m * TOK, (m + 1) * TOK)
                        nc.vector.tensor_tensor(
                            x_sb[:, ms], x_sb[:, ms], po[:], op=ADD)

                # ---------------- norm2 + MLP ----------------
                if DEBUG:
                    nc.sync.dma_start(
                        dbg["dbg_x1" if l == 0 else "dbg_x1_1"][:], x_sb[:])
                rmsnorm(f"m{l}")
                if DEBUG:
                    nc.sync.dma_start(
                        dbg["dbg_h2" if l == 0 else "dbg_h2_1"][:], h_sb[:])
                with (
                    tc.tile_pool(name=f"ps_d{l}", bufs=1, space="PSUM") as ppd,
                    tc.tile_pool(name=f"ps_gu{l}", bufs=2,
                                 space="PSUM") as ppg,
                ):
                    pd = ppd.tile([128, HT * TOK], F32, tag="pd")
                    # 8 independent 1KB accumulation regions share PSUM
                    # banks, and matmul start=True zeroes the whole bank:
                    # init by memset and accumulate with start=False.
                    nc.vector.memset(pd[:], 0.0)
                    NSTEP = FC * FM         # 32 global (fc, fm) steps
                    acts = {}

                    def emit_down(i):
                        fc, fk = i // FM, i % FM
                        wd_sb = mlp_w[fc % 2][2]
                        a = acts.pop(i)
                        for m in range(HT):
                            nc.tensor.matmul(
                                pd[:, m * TOK:(m + 1) * TOK],
                                wd_sb[:, (m * FM + fk) * 128:
                                      (m * FM + fk + 1) * 128],
                                a[:],
                                start=False, stop=(i == NSTEP - 1),
                                skip_group_check=True,
                            )

                    for i in range(NSTEP):
                        fc, fm = i // FM, i % FM
                        wg_sb, wu_sb, _ = mlp_w[fc % 2]
                        if i > 0:
                            # down for the previous step BEFORE the prefetch
                            # below recycles its weight buffer
                            emit_down(i - 1)
                        if fm == 0 and 1 <= fc < FC - 1:
                            # prefetch chunk fc+1 (chunks 0/1 preloaded)
                            mlp_w[(fc + 1) % 2] = load_mlp_chunk(l, fc + 1)
                        pg = ppg.tile([128, TOK], F32, tag="pg")
                        pu = ppg.tile([128, TOK], F32, tag="pu")
                        for kt in range(HT):
                            rhs = h_sb[:, kt * TOK:(kt + 1) * TOK]
                            nc.tensor.matmul(
                                pg[:],
                                wg_sb[:, (fm * HT + kt) * 128:
                                      (fm * HT + kt + 1) * 128],
                                rhs, start=(kt == 0), stop=(kt == HT - 1))
                            nc.tensor.matmul(
                                pu[:],
                                wu_sb[:, (fm * HT + kt) * 128:
                                      (fm * HT + kt + 1) * 128],
                                rhs, start=(kt == 0), stop=(kt == HT - 1))
                        sg = pact.tile([128, TOK], BF16, tag="sg")
                        nc.scalar.activation(
                            sg[:], pg[:], mybir.ActivationFunctionType.Silu)
                        a = pact.tile([128, TOK], BF16, tag="a", bufs=3)
                        nc.vector.tensor_tensor(a[:], sg[:], pu[:], op=MUL)
                        acts[i] = a
                    emit_down(NSTEP - 1)
                    if l + 1 < L:
                        # next layer's first two chunks (fetched during its
                        # attention phase; WAR-safe: emitted after all reads)
                        mlp_w = [load_mlp_chunk(l + 1, 0),
                                 load_mlp_chunk(l + 1, 1)]
                    for m in range(HT):
                        ms = slice(m * TOK, (m + 1) * TOK)
                        nc.vector.tensor_tensor(
                            x_sb[:, ms], x_sb[:, ms], pd[:, ms], op=ADD)
                if DEBUG and l == 0:
                    nc.sync.dma_start(dbg["dbg_x2"][:], x_sb[:])

            nc.sync.dma_start(xout[:], x_sb[:])
    return nc


def _get_nc():
    global _CACHED_NC
    if _CACHED_NC is None:
        _CACHED_NC = _build_nc()
    return _CACHED_NC


def _sel_matrix():
    s = np.zeros((16, MT * 128), np.float32)
    for m in range(MT):
        for j in range(2):
            s[2 * m + j, m * 128 + j * 64:m * 128 + (j + 1) * 64] = 1.0
    return s


def _host_prep(inputs):
    """Fold ln/scale into weights, build lhsT-layout replicated weight arrays
    (shared across cores), embed gather + per-core residual slices, rope
    tables.  Returns in_maps (list of dicts, one per core)."""
    ids = np.asarray(inputs["input_ids"])[0]          # [S] int32
    embed = np.asarray(inputs["embed"], np.float32)   # [V, H]
    x = embed[ids]                                    # [S, H]

    inv = 1.0 / (10000.0 ** (np.arange(0, D, 2, dtype=np.float32) / D))
    freqs = np.arange(S, dtype=np.float32)[:, None] * inv[None, :]  # [S,32]
    cosT = np.cos(freqs).T.astype(np.float32)   # [32, S]
    sinT = np.sin(freqs).T.astype(np.float32)
    cosF = np.tile(cosT, (2, 1))                            # [64, S]
    sinF = np.concatenate([-sinT, sinT], 0)                 # [64, S]

    def bf(a):
        return np.ascontiguousarray(a).astype(BF)

    scale = np.float32(1.0 / np.sqrt(D))
    shared = {}
    for l in range(L):
        ln1 = np.asarray(inputs["ln1"], np.float32)[l]
        ln2 = np.asarray(inputs["ln2"], np.float32)[l]
        wq = np.asarray(inputs["Wq"], np.float32)[l] * ln1[None, :] * scale
        wk = np.asarray(inputs["Wk"], np.float32)[l] * ln1[None, :]
        wv = np.asarray(inputs["Wv"], np.float32)[l] * ln1[None, :]
        wo = np.asarray(inputs["Wo"], np.float32)[l]
        wg = np.asarray(inputs["Wg"], np.float32)[l] * ln2[None, :]
        wu = np.asarray(inputs["Wu"], np.float32)[l] * ln2[None, :]
        wd = np.asarray(inputs["Wd"], np.float32)[l]

        # wkv: [128, (kt)*128]: rows=in-dims of kt, cols=[64 k | 64 v]
        kv = np.concatenate([wk, wv], 0).T                  # [H, 128]
        shared[f"wkv{l}"] = bf(
            kv.reshape(HT, 128, 128).transpose(1, 0, 2).reshape(128, -1))
        # wq: blocks (m*HT+kt): T[kt*128+r, m*128+c]
        T = wq.T                                            # [in, out]
        shared[f"wq{l}"] = bf(
            T.reshape(HT, 128, MT, 128).transpose(1, 2, 0, 3)
            .reshape(128, -1))
        # wo: blocks (m*HT+kt): rows=o-in dims kt, cols=H dims m
        T = wo.T                                            # [o-in, H]
        shared[f"wo{l}"] = bf(
            T.reshape(HT, 128, HT, 128).transpose(1, 2, 0, 3)
            .reshape(128, -1))
        # wg/wu chunks: blocks (fm*HT+kt): rows=in-dims kt, cols=FF dims
        for name, wt in (("wg", wg), ("wu", wu)):
            T = wt.T                                        # [H, FF]
            Tb = T.reshape(HT, 128, FC, FM, 128)
            for fc in range(FC):
                shared[f"{name}{l}_{fc}"] = bf(
                    Tb[:, :, fc].transpose(1, 2, 0, 3).reshape(128, -1))
        # wd chunks: blocks (m*FM+fk): rows=FF dims (fc,fk), cols=H dims m
        T = wd.T                                            # [FF, H]
        Tb = T.reshape(FC, FM, 128, HT, 128)
        for fc in range(FC):
            # (fk, r, m, c) -> (r, m, fk, c): block (m*FM+fk), partition r
            shared[f"wd{l}_{fc}"] = bf(
                Tb[fc].transpose(1, 2, 0, 3).reshape(128, -1))

    in_maps = []
    for c in range(NCORES):
        sl = slice(c * TOK, (c + 1) * TOK)
        xT = np.ascontiguousarray(x[sl].T)                  # [H, TOK]
        m = {
            "x0": np.ascontiguousarray(
                xT.reshape(HT, 128, TOK).transpose(1, 0, 2)
                .reshape(128, -1)),
            "cosk": bf(cosF[:, sl]),
            "sink": bf(sinF[:, sl]),
            "cosq": bf(np.tile(cosF[:, sl], (1, 8))),
            "sinq": bf(np.tile(sinF[:, sl], (1, 8))),
            "sel": _sel_matrix(),
        }
        m.update(shared)
        in_maps.append(m)
    return in_maps


def kernel(**inputs) -> np.ndarray:
    nc = _get_nc()
    in_maps = _host_prep(inputs)
    res = bass_utils.run_bass_kernel_spmd(
        nc, in_maps, core_ids=list(range(NCORES))
    )
    out = np.empty((1, S, H), np.float32)
    for c in range(NCORES):
        xo = res.results[c]["xout"]            # [128, HT*TOK]
        out[0, c * TOK:(c + 1) * TOK, :] = (
            xo.reshape(128, HT, TOK).transpose(1, 0, 2)
            .reshape(H, TOK).T)
    return out
